# revision 26
# baseline (speedup 1.0000x reference)
"""Causal multi-head attention (B=2, S=2048, D=1024, H=16) on 8 trn2 cores.

Sharding: core c computes heads {2c, 2c+1} of BOTH batches (tensor parallel
over heads). All matmul operands are bf16 (psum accumulation fp32).

Pipeline: 4 stages (stage s = seq quarter s). Stage s emits, interleaved so
the PE queue never head-blocks on ScalarE exp:
  - attention for q-block s (scores^T = K Q^T row-tiled head pairs on PE,
    exp on ScalarE, narrow causal band masks on gpsimd, A^T V on PE with a
    fused ones-column emitting softmax denominators),
  - projection MM chains for seq quarter s+1 (Q^T/K^T transposed layout,
    V natural layout packed 4-chunks-per-psum-bank),
  - output projection for quarter s-1 (after that quarter's AllToAll).
After each stage: per-quarter 8-way AllToAll (256KB/core) redistributes
head outputs so EVERY core gets one 128-row strip of each quarter
(out-projection load spread evenly; only the last quarter's a2a+proj is
exposed). Host assembles the 8 x 4 strips into (2, 2048, 1024).
"""

import sys

for _p in ("/opt/trn_rl_repo", "/opt/pypackages"):
    if _p not in sys.path:
        sys.path.insert(0, _p)

import ml_dtypes
import numpy as np

import concourse.bass as bass
import concourse.mybir as mybir
import concourse.tile as tile
from concourse import bacc
from concourse.bass_utils import run_bass_kernel_spmd

B = 2
S = 2048
D = 1024
H = 16
DH = 64
NCORES = 8
SB = 512           # q block (matmul moving dim)
KC = 128           # k chunk (contraction tile)
NSB = S // SB      # 4 q-blocks / seq quarters
NKC = S // KC      # 16 k-chunks
NDC = D // KC      # 8 contraction chunks for the projections

BF16 = ml_dtypes.bfloat16

_compiled = None


def _interleave(main, fillers, late_fillers=()):
    """Emission-order weave: spread `fillers` evenly among `main` units,
    `late_fillers` evenly through the second half."""
    n = max(len(main), 1)
    slots = [[] for _ in range(n + 1)]
    nf = len(fillers)
    for i, f in enumerate(fillers):
        slots[min(((i + 1) * n) // (nf + 1), n)].append(f)
    nl = len(late_fillers)
    for i, f in enumerate(late_fillers):
        pos = n // 2 + ((i + 1) * (n - n // 2)) // (nl + 1)
        slots[min(pos, n)].append(f)
    for i, m in enumerate(main):
        for f in slots[i]:
            f()
        m()
    for f in slots[n]:
        f()


def _build():
    f32 = mybir.dt.float32
    bf16 = mybir.dt.bfloat16
    nc = bacc.Bacc(None, target_bir_lowering=False)

    # host-blocked inputs, laid out so every DMA's DRAM access pattern is
    # PLAIN row-major (permuted APs classify as DMA transposes and get
    # serialized against collectives by the scheduler): partition dim
    # before the contraction-chunk dim.
    xt = nc.declare_dram_parameter("xt", [B, NSB, KC, NDC, SB], bf16, isOutput=False)
    wqk = nc.declare_dram_parameter("wqk", [KC, NDC, 2 * KC], bf16, isOutput=False)
    wv = nc.declare_dram_parameter("wv", [KC, NDC, 2 * DH], bf16, isOutput=False)
    wout = nc.declare_dram_parameter("wout", [KC, NDC, D], bf16, isOutput=False)
    bqk = nc.declare_dram_parameter("bqk", [KC, 2], f32, isOutput=False)
    bv4 = nc.declare_dram_parameter("bv4", [4 * 2 * DH], f32, isOutput=False)
    bo = nc.declare_dram_parameter("bo", [D], f32, isOutput=False)
    out_ext = nc.declare_dram_parameter("out", [NSB, KC, D], f32, isOutput=True)

    # Per-quarter AllToAll staging. Block i of quarter q = (batch i//4,
    # seq rows 512q + 128*(i%4)): core i ends up owning that 128-row strip.
    a2a_in = [nc.dram_tensor(f"a2a_in{q}", [NCORES, KC, KC], bf16)
              for q in range(NSB)]
    a2a_out = [nc.dram_tensor(f"a2a_out{q}", [NCORES, KC, KC], bf16)
               for q in range(NSB)]

    with tile.TileContext(nc) as tc:
        with (
            tc.tile_pool(name="wts", bufs=1) as wp,
            tc.tile_pool(name="qkv", bufs=1) as qkvp,
            tc.tile_pool(name="xbuf", bufs=20) as xp,
            tc.tile_pool(name="pbuf", bufs=1) as pb,
            tc.tile_pool(name="obuf", bufs=1) as op,
            tc.tile_pool(name="recv", bufs=1) as rp,
            tc.tile_pool(name="misc", bufs=1) as mp,
            tc.tile_pool(name="evict", bufs=1) as ep,
            tc.tile_pool(name="psum_pj", bufs=1, space="PSUM") as pp,
            tc.tile_pool(name="psum_sc", bufs=1, space="PSUM") as pa,
            tc.tile_pool(name="psum_av", bufs=1, space="PSUM") as pv,
        ):
            # ---- persistent weights (single batched DMA each: a dma_start
            # costs ~0.6us of sequencer issue time, so batch aggressively) --
            wqk_t = wp.tile([KC, NDC * 2 * KC], bf16, tag="wqk", name="wqk_t")
            wv_t = wp.tile([KC, NDC * 2 * DH], bf16, tag="wv", name="wv_t")
            wout_t = wp.tile([KC, NDC * D], bf16, tag="wo", name="wout_t")
            nc.sync.dma_start(
                out=wqk_t[:].rearrange("p (k c) -> p k c", c=2 * KC),
                in_=wqk[:],
            )
            nc.gpsimd.dma_start(
                out=wv_t[:].rearrange("p (k c) -> p k c", c=2 * DH),
                in_=wv[:],
            )

            # ---- small constants -----------------------------------------
            bqk_t = mp.tile([KC, 2], f32, tag="bqk", name="bqk_t")
            nc.sync.dma_start(out=bqk_t[:], in_=bqk[:])
            bv_row = mp.tile([1, 4 * 2 * DH], f32, tag="bv_row")
            nc.sync.dma_start(out=bv_row[:], in_=bv4.rearrange("(o f) -> o f", o=1))
            bv_bc4 = mp.tile([KC, 4 * 2 * DH], f32, tag="bv_bc4")
            bo_row = mp.tile([1, D], f32, tag="bo_row")
            nc.sync.dma_start(out=bo_row[:], in_=bo.rearrange("(o f) -> o f", o=1))
            bo_bc = mp.tile([KC, D], f32, tag="bo_bc")

            def broadcast_consts():
                # emitted after the stage-0 x^T DMA issues so these gpsimd
                # ops don't delay them (issue queues are FIFO)
                nc.gpsimd.partition_broadcast(out_ap=bv_bc4[:], in_ap=bv_row[:])
                nc.gpsimd.partition_broadcast(out_ap=bo_bc[:], in_ap=bo_row[:])

            # ---- persistent activations ----------------------------------
            # QQ[p][s]: rows 0:64 = Q^T of head 2c, rows 64:128 = head 2c+1
            QQ = [[qkvp.tile([KC, SB], bf16, tag=f"QQ{p}_{s}", name=f"QQ{p}_{s}")
                   for s in range(NSB)] for p in range(2)]
            KK = [[qkvp.tile([KC, SB], bf16, tag=f"KK{p}_{s}", name=f"KK{p}_{s}")
                   for s in range(NSB)] for p in range(2)]
            # V[2p+hh][s]: [128, 4*65]; chunk sc at cols sc*65..+64; col 64: 1.0
            NCS = SB // KC
            V = [[qkvp.tile([KC, NCS * (DH + 1)], bf16, tag=f"V{v}_{s}",
                            name=f"V{v}_{s}")
                  for s in range(NSB)] for v in range(4)]
            for v in range(4):
                for s in range(NSB):
                    vv = V[v][s][:].rearrange("p (k c) -> p k c", c=DH + 1)
                    nc.vector.memset(vv[:, :, DH:DH + 1], 1.0)

            # ---- unit builders -------------------------------------------
            def alloc_xt(sblk):
                """x^T tiles for quarter sblk: one [128, 8*512] tile per
                batch (all 8 contraction chunks side by side, ONE DMA)."""
                return [xp.tile([KC, NDC * SB], bf16, tag="xt", bufs=4,
                                name=f"x{sblk}_{bb}") for bb in range(B)]

            def load_xt_unit(sblk, xtiles):
                def u():
                    for bb in range(B):
                        eng = nc.sync if bb == 0 else nc.gpsimd
                        eng.dma_start(
                            out=xtiles[bb][:].rearrange("p (k c) -> p k c", c=SB),
                            in_=xt[bb, sblk],
                        )
                return u

            def proj_units(sblk, xtiles):
                """6 PE chain units projecting seq quarter `sblk`."""
                units = []

                def xs(bb, k):
                    return xtiles[bb][:, k * SB:(k + 1) * SB]

                for bb in range(B):
                    for m in range(2):
                        def qk_unit(bb=bb, m=m):
                            ps = pp.tile([KC, SB], f32, tag="ps_pj", bufs=2)
                            for k in range(NDC):
                                nc.tensor.matmul(
                                    ps[:],
                                    wqk_t[:, (2 * k + m) * KC:(2 * k + m + 1) * KC],
                                    xs(bb, k),
                                    start=(k == 0),
                                    stop=(k == NDC - 1),
                                )
                            dest = (QQ if m == 0 else KK)[bb][sblk]
                            nc.vector.tensor_scalar_add(
                                dest[:], ps[:], bqk_t[:, m:m + 1])
                        units.append(qk_unit)

                    def v_unit(bb=bb):
                        # natural layout: lhsT = x^T chunk (stationary),
                        # rhs = Wv [128, 128]; 4 sc regions in one psum bank
                        psv = pp.tile([KC, SB], f32, tag="ps_pj", bufs=2)
                        for sc in range(NCS):
                            for k in range(NDC):
                                nc.tensor.matmul(
                                    psv[:, sc * KC:(sc + 1) * KC],
                                    xs(bb, k)[:, sc * KC:(sc + 1) * KC],
                                    wv_t[:, 2 * k * DH:2 * (k + 1) * DH],
                                    start=(k == 0),
                                    stop=(k == NDC - 1),
                                )
                        ps3 = psv[:].rearrange("p (k c) -> p k c", c=2 * DH)
                        bv3 = bv_bc4[:].rearrange("p (k c) -> p k c", c=2 * DH)
                        for hh in range(2):
                            vd = V[2 * bb + hh][sblk][:].rearrange(
                                "p (k c) -> p k c", c=DH + 1)
                            nc.vector.tensor_add(
                                vd[:, :, 0:DH],
                                ps3[:, :, hh * DH:(hh + 1) * DH],
                                bv3[:, :, hh * DH:(hh + 1) * DH],
                            )
                    units.append(v_unit)
                return units

            def outproj_units(q):
                """recv + 2 PE chain units projecting my strip of quarter q."""
                recv = rp.tile([KC, NCORES * KC], bf16, tag="recv", bufs=2,
                               name=f"recv{q}")

                def recv_unit():
                    # quarters 0-2: sync queue only (a blocked recv on the
                    # scalar queue would head-block exp). Quarter 3 runs when
                    # everything else is drained: spread across 4 queues.
                    if q < NSB - 1:
                        engs = [nc.sync] * NCORES
                    else:
                        engs = [nc.sync, nc.scalar, nc.gpsimd, nc.sync,
                                nc.scalar, nc.gpsimd, nc.sync, nc.scalar]
                    for s in range(NCORES):
                        engs[s].dma_start(
                            out=recv[:, s * KC:(s + 1) * KC], in_=a2a_out[q][s])

                units = [recv_unit]
                ot = ep.tile([KC, D], f32, tag="ot", bufs=2, name=f"ot{q}")
                for nb in range(2):
                    def o_unit(nb=nb):
                        ps = pp.tile([KC, SB], f32, tag="ps_pj", bufs=2)
                        for k in range(NDC):
                            nc.tensor.matmul(
                                ps[:],
                                recv[:, k * KC:(k + 1) * KC],
                                wout_t[:, k * D + nb * SB:k * D + (nb + 1) * SB],
                                start=(k == 0),
                                stop=(k == NDC - 1),
                            )
                        nc.vector.tensor_add(
                            ot[:, nb * SB:(nb + 1) * SB], ps[:],
                            bo_bc[:, nb * SB:(nb + 1) * SB])
                        if nb == 1:
                            nc.sync.dma_start(out=out_ext[q], in_=ot[:])
                    units.append(o_unit)
                return units

            # ---- attention machinery -------------------------------------
            # P/pos tiles created lazily at emission time so pool tag
            # rotation follows true program order (qblk-3 p0 chunks kc<8 are
            # emitted early, during stage 2, and get dedicated tags).
            P_reg = {}
            pos_reg = {}

            def get_P(qblk, p, kc):
                key = (qblk, p, kc)
                if key not in P_reg:
                    if qblk == 3 and p == 0 and kc < 8:
                        tag, bufs = f"P3e_{kc}", 1
                    else:
                        tag, bufs = f"P{kc}", (2 if kc < 12 else 1)
                    P_reg[key] = pb.tile([KC, 2 * SB], bf16, tag=tag,
                                         bufs=bufs, name=f"P_{qblk}_{p}_{kc}")
                return P_reg[key]

            def get_pos(qblk, p, hh):
                key = (qblk, p, hh)
                if key not in pos_reg:
                    pos_reg[key] = pv.tile([DH + 1, SB], f32, tag=f"pos{hh}",
                                           bufs=1, name=f"pos{hh}_{p}_{qblk}")
                return pos_reg[key]

            def chunk_unit(qblk, p, kc):
                def u():
                    d = kc - 4 * qblk
                    c0 = KC * max(d, 0)
                    P = get_P(qblk, p, kc)
                    ps = pa.tile([KC, 2 * SB], f32, tag="ps_s", bufs=2)
                    for hh in range(2):  # row-tiled head pair
                        r0 = hh * DH
                        nc.tensor.matmul(
                            ps[:, hh * SB + c0:(hh + 1) * SB],
                            KK[p][kc // 4][r0:r0 + DH,
                                           (kc % 4) * KC:(kc % 4 + 1) * KC],
                            QQ[p][qblk][r0:r0 + DH, c0:SB],
                            start=True,
                            stop=True,
                        )
                    ps3 = ps[:].rearrange("p (h f) -> p h f", h=2)
                    pd3 = P[:].rearrange("p (h f) -> p h f", h=2)
                    nc.scalar.activation(
                        pd3[:, :, c0:SB],
                        ps3[:, :, c0:SB],
                        mybir.ActivationFunctionType.Exp,
                        scale=1.0 / float(np.sqrt(DH)),
                    )
                    if d >= 0:  # diagonal chunk: zero band where k > q
                        nc.gpsimd.affine_select(
                            out=pd3[:, :, c0:c0 + KC],
                            in_=pd3[:, :, c0:c0 + KC],
                            pattern=[[0, 2], [1, KC]],
                            compare_op=mybir.AluOpType.is_ge,
                            fill=0.0,
                            base=0,
                            channel_multiplier=-1,
                        )
                return u

            def av_emit(qblk, p, kc):
                d = kc - 4 * qblk
                c0 = KC * max(d, 0)
                nkc = 4 * (qblk + 1)
                P = get_P(qblk, p, kc)
                for hh in range(2):
                    nc.tensor.matmul(
                        get_pos(qblk, p, hh)[:, c0:SB],
                        V[2 * p + hh][kc // 4][:, (kc % 4) * (DH + 1):
                                               (kc % 4 + 1) * (DH + 1)],
                        P[:, hh * SB + c0:(hh + 1) * SB],
                        start=(kc == 0),
                        stop=(kc == nkc - 1),
                    )

            def tail_unit(qblk, p):
                def u():
                    O = op.tile([KC, SB], bf16, tag=f"O{p}", bufs=2,
                                name=f"O{p}_{qblk}")
                    for hh in range(2):
                        pos = get_pos(qblk, p, hh)
                        den = ep.tile([1, SB], f32, tag="den", bufs=2)
                        nc.vector.tensor_copy(den[:], pos[DH:DH + 1, :])
                        rden = ep.tile([1, SB], f32, tag="rden", bufs=2)
                        nc.vector.reciprocal_approx_fast(out=rden[:], in_=den[:])
                        rden_bc = ep.tile([DH, SB], f32, tag="rbc", bufs=2)
                        nc.gpsimd.partition_broadcast(
                            out_ap=rden_bc[:], in_ap=rden[:])
                        nc.vector.tensor_mul(
                            O[hh * DH:(hh + 1) * DH, :],
                            pos[0:DH, :],
                            rden_bc[:],
                        )
                    # per-block plain DMAs (block dim outer in DRAM; a single
                    # strided DMA would classify as a transpose and serialize
                    # against the collectives)
                    for j in range(4):
                        eng = nc.sync if j % 2 == 0 else nc.scalar
                        eng.dma_start(
                            out=a2a_in[qblk][4 * p + j],
                            in_=O[:, j * KC:(j + 1) * KC],
                        )
                return u

            def attn_p_units(qblk, p, early_done=()):
                """Chunk units (minus early_done) with AV matmuls woven in at
                a matched rate (AV lags 2 chunks so exp stays ahead), then the
                normalize/staging tail."""
                nkc = 4 * (qblk + 1)
                chunks = [kc for kc in range(nkc) if kc not in early_done]
                units = []
                ai = 0
                for i, kc in enumerate(chunks):
                    units.append(chunk_unit(qblk, p, kc))
                    target = ((i + 1) * nkc) // len(chunks)
                    emit = []
                    while ai < min(target, nkc) and (
                            ai in early_done or ai <= kc - 2):
                        emit.append(ai)
                        ai += 1
                    if emit:
                        def avs(emit=emit):
                            for a in emit:
                                av_emit(qblk, p, a)
                        units.append(avs)

                def drain(ai0=ai):
                    for a in range(ai0, nkc):
                        av_emit(qblk, p, a)
                units.append(drain)
                units.append(tail_unit(qblk, p))
                return units

            # ---- the pipeline --------------------------------------------
            # stage s: attention for q-block s, interleaved with projection
            # chains for quarter s+1 and (from stage 2) the out-projection of
            # quarter s-2 (one extra stage of slack absorbs a2a peer skew).
            # Stage 2 additionally pre-runs scores+exp for qblk3/p0 kc<8 to
            # level ScalarE load between stages 2 and 3.
            EARLY3 = tuple(range(8))
            xtiles = alloc_xt(0)
            p0units = proj_units(0, xtiles)
            # batch 0's x first; batch 1's load issued behind it so bb0's
            # first projection chain isn't starved by bb1's transfer
            nc.sync.dma_start(
                out=xtiles[0][:].rearrange("p (k c) -> p k c", c=SB),
                in_=xt[0, 0],
            )
            broadcast_consts()
            for u in p0units[:3]:
                u()
            nc.gpsimd.dma_start(
                out=xtiles[1][:].rearrange("p (k c) -> p k c", c=SB),
                in_=xt[1, 0],
            )
            for u in p0units[3:]:
                u()
            for stage in range(NSB):
                mains = []
                for p in range(2):
                    early = EARLY3 if (stage == 3 and p == 0) else ()
                    mains += attn_p_units(stage, p, early)
                if stage == 2:
                    mains += [chunk_unit(3, 0, kc) for kc in EARLY3]
                fillers = []
                late = []
                if stage < NSB - 1:
                    xtiles = alloc_xt(stage + 1)
                    if stage == 0:
                        # stage 0 has ScalarE slack: keep proj(1) chains in
                        # the second half so the PE never waits on the xt(1)
                        # transfer that was just issued
                        fillers = [load_xt_unit(stage + 1, xtiles)]
                        late = proj_units(stage + 1, xtiles)
                    else:
                        fillers = [load_xt_unit(stage + 1, xtiles)]
                        fillers += proj_units(stage + 1, xtiles)
                if stage == 1:
                    def wout_loader():
                        nc.sync.dma_start(
                            out=wout_t[:].rearrange("p (k c) -> p k c", c=D),
                            in_=wout[:],
                        )
                    fillers = [wout_loader] + fillers
                if stage >= 2:
                    late = outproj_units(stage - 2)
                if stage == NSB - 1:
                    # quarter-2 out-projection also inside stage 3 (any DMA
                    # issued after the last collective's trigger serializes
                    # behind that collective's completion)
                    late = late + outproj_units(NSB - 2)
                _interleave(mains, fillers, late)
                nc.gpsimd.collective_compute(
                    "AllToAll",
                    mybir.AluOpType.bypass,
                    replica_groups=[[0, 1, 2, 3, 4, 5, 6, 7]],
                    ins=[a2a_in[stage][:]],
                    outs=[a2a_out[stage][:]],
                )
            for u in outproj_units(NSB - 1):
                u()

    nc.compile()
    return nc


def _get_program():
    global _compiled
    if _compiled is None:
        _compiled = _build()
    return _compiled


def _shard_inputs(x, Wqkv, bqkv, Wout, bout):
    """Build the 8 per-core input maps (all host-side numpy, bf16 data)."""
    x = np.asarray(x, dtype=np.float32)
    Wqkv = np.asarray(Wqkv, dtype=np.float32)
    bqkv = np.asarray(bqkv, dtype=np.float32)
    Wout = np.asarray(Wout, dtype=np.float32)
    bout = np.ascontiguousarray(np.asarray(bout, dtype=np.float32))

    Wq = Wqkv[:, 0 * D:1 * D]
    Wk = Wqkv[:, 1 * D:2 * D]
    Wv_full = Wqkv[:, 2 * D:3 * D]
    bq = bqkv[0 * D:1 * D]
    bk = bqkv[1 * D:2 * D]
    bv_full = bqkv[2 * D:3 * D]

    # shared across all cores; layouts keep the SBUF partition dim (p)
    # ahead of the contraction-chunk dim (k) so device DMAs are plain
    xt = np.ascontiguousarray(
        x.transpose(0, 2, 1)                      # [B, D, S]
         .reshape(B, NDC, KC, NSB, SB).transpose(0, 3, 2, 1, 4)
    ).astype(BF16)
    wout_b = np.ascontiguousarray(
        Wout.reshape(NDC, KC, D).transpose(1, 0, 2)).astype(BF16)

    in_maps = []
    for c in range(NCORES):
        ha, hb = 2 * c, 2 * c + 1
        wqk_c = np.ascontiguousarray(np.concatenate(
            [Wq[:, ha * DH:(ha + 1) * DH], Wq[:, hb * DH:(hb + 1) * DH],
             Wk[:, ha * DH:(ha + 1) * DH], Wk[:, hb * DH:(hb + 1) * DH]],
            axis=1).reshape(NDC, KC, 2 * KC).transpose(1, 0, 2)).astype(BF16)
        bqk_c = np.ascontiguousarray(np.concatenate(
            [bq[ha * DH:(ha + 1) * DH], bq[hb * DH:(hb + 1) * DH],
             bk[ha * DH:(ha + 1) * DH], bk[hb * DH:(hb + 1) * DH]]
        ).reshape(2, KC).T)
        wv_c = np.ascontiguousarray(np.concatenate(
            [Wv_full[:, ha * DH:(ha + 1) * DH],
             Wv_full[:, hb * DH:(hb + 1) * DH]],
            axis=1).reshape(NDC, KC, 2 * DH).transpose(1, 0, 2)).astype(BF16)
        bv_c = np.concatenate(
            [bv_full[ha * DH:(ha + 1) * DH], bv_full[hb * DH:(hb + 1) * DH]])
        bv4_c = np.ascontiguousarray(np.tile(bv_c, SB // KC))
        in_maps.append({
            "xt": xt, "wqk": wqk_c, "wv": wv_c, "wout": wout_b,
            "bqk": bqk_c, "bv4": bv4_c, "bo": bout,
        })
    return in_maps


def run(inputs, trace=False, trace_kwargs=None):
    nc = _get_program()
    in_maps = _shard_inputs(**inputs)
    res = run_bass_kernel_spmd(
        nc, in_maps, list(range(NCORES)), trace=trace,
        **(trace_kwargs or {}),
    )
    out = np.empty((B, S, D), dtype=np.float32)
    for c in range(NCORES):
        b = c // 4
        for q in range(NSB):
            r0 = SB * q + KC * (c % 4)
            out[b, r0:r0 + KC, :] = res.results[c]["out"][q]
    return out, res


def kernel(**inputs):
    out, _ = run(inputs)
    return out
